# revision 2
# baseline (speedup 1.0000x reference)
"""Gcs pairwise-distance loss kernel (final, v12) for Trainium2 (Bass/Tile), 8-core SPMD.

Math: with d = pred - truth viewed as [128, 512] (partition p = 4b + k,
k in {0,1} = X column chunks, {2,3} = Y) and rs[p] = rc[p] + rc[p^1] the
full group-row sum:

    sumsq[m,j]/4096 = sum_{p%4==m} (d^2/4 - (2/4096) rs d)  +  S2_g/4096
    loss            = sum_{m,j} sqrt(sumsq[m,j]/4096)   (folds the /64)

v11 (vs v9): dsq_a moves to ScalarE as a Square activation (same ACT
table set as Sqrt), so DVE runs td_b as soon as half B lands instead of
after dsq_a; dsq_b stays on DVE for its cheap accumulator read.  Each column half arrives as ONE packed
[128,512] DMA (pred_h | truth_h side by side) — half A on the SP queue,
half B on the ACT queue, so the two descriptor pipelines and completion
sems process in parallel.  The d^2 matmuls run per column half in ONE PSUM
accumulation group (start only on the first matmul — the PSUM start reset
is bank-wide); the -2rs*d matmuls close the group.  The rs chain is fused
(one [128,2] shuffle + one accumulating stt + one stt for lhsT2).  The S2
bias merge rides the idle ScalarE as a Copy activation with accum_out
(both ACT tables are loaded up front regardless).  The Sqrt's accum_out
dsums[4,1] is DMA'd out; the host gather adds 4 floats.

Every core computes the full replicated result; core 0's output is returned.
"""

import numpy as np

_CACHE = {}


def _build_nc():
    import concourse.tile as tile
    from concourse import bacc, mybir

    f32 = mybir.dt.float32
    bf16 = mybir.dt.bfloat16
    i32 = mybir.dt.int32
    Alu = mybir.AluOpType
    Act = mybir.ActivationFunctionType
    nc = bacc.Bacc("TRN2", target_bir_lowering=False, debug=False)
    inA = nc.dram_tensor("inA", [128, 512], bf16, kind="ExternalInput").ap()
    inB = nc.dram_tensor("inB", [128, 512], bf16, kind="ExternalInput").ap()
    out = nc.dram_tensor("out", [4, 1], f32, kind="ExternalOutput").ap()

    tinA = nc.alloc_sbuf_tensor("tinA", [128, 512], bf16).ap()
    tinB = nc.alloc_sbuf_tensor("tinB", [128, 512], bf16).ap()
    td = nc.alloc_sbuf_tensor("td", [128, 512], bf16).ap()
    dsq = nc.alloc_sbuf_tensor("dsq", [128, 512], bf16).ap()
    rc2 = nc.alloc_sbuf_tensor("rc2", [128, 2], f32).ap()
    rcs2 = nc.alloc_sbuf_tensor("rcs2", [128, 2], f32).ap()
    junk2 = nc.alloc_sbuf_tensor("junk2", [128, 2], f32).ap()
    m2rs = nc.alloc_sbuf_tensor("m2rs", [128, 1], f32).ap()
    sq2 = nc.alloc_sbuf_tensor("sq2", [128, 2], f32).ap()
    ti1 = nc.alloc_sbuf_tensor("ti1", [128, 4], i32).ap()
    ti1b = nc.alloc_sbuf_tensor("ti1b", [128, 4], i32).ap()
    mask01 = nc.alloc_sbuf_tensor("mask01", [128, 4], bf16).ap()
    mask01c = nc.alloc_sbuf_tensor("mask01c", [128, 4], bf16).ap()
    lhsT2 = nc.alloc_sbuf_tensor("lhsT2", [128, 4], bf16).ap()
    ti2 = nc.alloc_sbuf_tensor("ti2", [128, 4], i32).ap()
    ti2b = nc.alloc_sbuf_tensor("ti2b", [128, 4], i32).ap()
    maskS = nc.alloc_sbuf_tensor("maskS", [128, 4], f32).ap()
    ones4 = nc.alloc_sbuf_tensor("ones4", [4, 1], f32).ap()
    warm = nc.alloc_sbuf_tensor("warm", [4, 1], f32).ap()
    junkb = nc.alloc_sbuf_tensor("junkb", [4, 2], f32).ap()
    biasb = nc.alloc_sbuf_tensor("biasb", [4, 1], f32).ap()
    dist = nc.alloc_sbuf_tensor("dist", [4, 512], f32).ap()
    dsums = nc.alloc_sbuf_tensor("dsums", [4, 1], f32).ap()

    main = nc.alloc_psum_tensor("main", [4, 512], f32).ap()
    biasK2 = nc.alloc_psum_tensor("biasK2", [4, 2], f32).ap()

    tin0a, tin1a = tinA[:, 0:256], tinA[:, 256:512]
    tin0b, tin1b = tinB[:, 0:256], tinB[:, 256:512]
    td_a, td_b = td[:, 0:256], td[:, 256:512]
    dsq_a, dsq_b = dsq[:, 0:256], dsq[:, 256:512]
    rc_a, rc_b = rc2[:, 0:1], rc2[:, 1:2]
    sq_a, sq_b = sq2[:, 0:1], sq2[:, 1:2]
    main_a, main_b = main[:, 0:256], main[:, 256:512]

    with tile.TileContext(nc) as tc:
        # ---- packed input DMAs (pred_h | truth_h), A then B, SP queue ----
        with tc.high_priority():
            nc.sync.dma_start(tinA, inA)
            nc.scalar.dma_start(tinB, inB)

        # ---- dependency-free dummy Sqrt pulls the ACT table loads early ----
        nc.gpsimd.memset(ones4, 1.0)
        nc.scalar.activation(warm, ones4, Act.Sqrt)

        # ---- on-chip constants (under the DMA shadow) ----
        nc.gpsimd.iota(ti1, pattern=[[-1, 4]], base=0, channel_multiplier=1)
        nc.vector.tensor_scalar(ti1b, ti1, 3, None, op0=Alu.bitwise_and)
        # mask01[p,m] = 1.0 (bf16) iff p % 4 == m  -- dsq matmul lhsT
        nc.vector.tensor_scalar(mask01, ti1b, 0, None, op0=Alu.is_equal)
        # mask01c[p,m] = -2/4096 at the same positions -- lhsT2 seed
        nc.vector.tensor_scalar(mask01c, ti1b, 0, -2.0 / 4096.0,
                                op0=Alu.is_equal, op1=Alu.mult)
        nc.gpsimd.iota(ti2, pattern=[[-2, 2], [0, 2]], base=0,
                       channel_multiplier=1)
        nc.vector.tensor_scalar(ti2b, ti2, 1, 1, op0=Alu.arith_shift_right,
                                op1=Alu.bitwise_and)
        # maskS[p,m] = 1/1024 (f32) iff (p>>1)&1 == m//2  -- S2 group lhsT
        nc.vector.tensor_scalar(maskS, ti2b, 0, 1.0 / 1024.0,
                                op0=Alu.is_equal, op1=Alu.mult)

        # ---- per-half d = pred - truth; dsq_a on ScalarE (Square of
        #      0.5*d = d^2/4), dsq_b on DVE ----
        nc.vector.scalar_tensor_tensor(
            out=td_a, in0=tin0a, scalar=1.0, in1=tin1a,
            op0=Alu.mult, op1=Alu.subtract, accum_out=rc_a,
        )
        nc.scalar.activation(dsq_a, td_a, Act.Square, scale=0.5,
                             accum_out=sq_a)
        nc.vector.scalar_tensor_tensor(
            out=td_b, in0=tin0b, scalar=1.0, in1=tin1b,
            op0=Alu.mult, op1=Alu.subtract, accum_out=rc_b,
        )
        nc.vector.scalar_tensor_tensor(
            out=dsq_b, in0=td_b, scalar=0.25, in1=td_b,
            op0=Alu.mult, op1=Alu.mult, accum_out=sq_b,
        )

        # ---- rs chain (fused): rs[p] = rc_a+rc_b+rc_a[p^1]+rc_b[p^1] ----
        nc.vector.stream_shuffle(rcs2, rc2, mask=[i ^ 1 for i in range(32)])
        nc.vector.scalar_tensor_tensor(
            out=junk2, in0=rc2, scalar=1.0, in1=rcs2,
            op0=Alu.mult, op1=Alu.add, accum_out=m2rs,
        )
        # lhsT2[p,m] = (-2/4096) * rs[p] at group rows (bf16)
        nc.vector.scalar_tensor_tensor(
            out=lhsT2, in0=mask01c, scalar=m2rs, in1=mask01,
            op0=Alu.mult, op1=Alu.mult,
        )

        # ---- PE: one PSUM group in the main bank (start reset is
        #      bank-wide): d^2 halves open it, -2rs*d halves close it ----
        nc.tensor.matmul(main_a, mask01, dsq_a, start=True, stop=False,
                         skip_group_check=True)
        nc.tensor.matmul(main_b, mask01, dsq_b, start=False, stop=False,
                         skip_group_check=True)
        nc.tensor.matmul(biasK2, maskS, sq2, start=True, stop=True)
        nc.tensor.matmul(main_a, lhsT2, td_a, start=False, stop=False,
                         skip_group_check=True)
        nc.tensor.matmul(main_b, lhsT2, td_b, start=False, stop=True,
                         skip_group_check=True)

        # ---- biasb[m] = S2_g/4096 on the idle ScalarE (Copy + accum) ----
        nc.scalar.activation(junkb, biasK2, Act.Copy, accum_out=biasb)

        # ---- dist = sqrt(main + S2_g/4096); accum = row sums ----
        nc.scalar.activation(dist, main, Act.Sqrt, bias=biasb,
                             scale=1.0, accum_out=dsums)

        # ---- ship the 4 partial sums; host gather adds them ----
        nc.sync.dma_start(out, dsums)

    nc.compile()
    return nc


def _get():
    if "nc" not in _CACHE:
        _CACHE["nc"] = _build_nc()
    return _CACHE["nc"]


def _in_map(pred, truth):
    import ml_dtypes

    nc = _get()
    p = np.asarray(pred, dtype=np.float32).reshape(128, 512).astype(ml_dtypes.bfloat16)
    t = np.asarray(truth, dtype=np.float32).reshape(128, 512).astype(ml_dtypes.bfloat16)
    inA = np.concatenate([p[:, 0:256], t[:, 0:256]], axis=1)
    inB = np.concatenate([p[:, 256:512], t[:, 256:512]], axis=1)
    return nc, {
        "inA": np.ascontiguousarray(inA),
        "inB": np.ascontiguousarray(inB),
    }


def kernel(pred, truth) -> np.ndarray:
    from concourse.bass_utils import run_bass_kernel_spmd

    nc, in_map = _in_map(pred, truth)
    res = run_bass_kernel_spmd(
        nc, [dict(in_map) for _ in range(8)], core_ids=list(range(8))
    )
    total = res.results[0]["out"].astype(np.float64).sum()
    return np.asarray(total, dtype=np.float32).reshape(())


# revision 3
# speedup vs baseline: 1.0340x; 1.0340x over previous
"""Gcs pairwise-distance loss kernel (final, v15) for Trainium2 (Bass/Tile), 8-core SPMD.

Math: with d = pred - truth viewed as [128, 512] (partition p = 4b + k,
k in {0,1} = X column chunks, {2,3} = Y) and rs[p] = rc[p] + rc[p^1] the
full group-row sum:

    sumsq[m,j]/4096 = sum_{p%4==m} (d^2/4 - (2/4096) rs d)  +  S2_g/4096
    loss            = sum_{m,j} sqrt(sumsq[m,j]/4096)   (folds the /64)

v11 (vs v9): dsq_a moves to ScalarE as a Square activation (same ACT
table set as Sqrt), so DVE runs td_b as soon as half B lands instead of
after dsq_a; dsq_b stays on DVE for its cheap accumulator read.  Each column half arrives as ONE packed
[128,512] DMA (pred_h | truth_h side by side) — half A on the SP queue,
half B on the ACT queue, so the two descriptor pipelines and completion
sems process in parallel.  The d^2 matmuls run per column half in ONE PSUM
accumulation group (start only on the first matmul — the PSUM start reset
is bank-wide); the -2rs*d matmuls close the group.  The rs chain is fused
(one [128,2] shuffle + one accumulating stt + one stt for lhsT2).  The S2
bias merge rides the idle ScalarE as a Copy activation with accum_out
(both ACT tables are loaded up front regardless).  The Sqrt's accum_out
dsums[4,1] is DMA'd out; the host gather adds 4 floats.

Every core computes the full replicated result; core 0's output is returned.
"""

import numpy as np

_CACHE = {}


def _build_nc():
    import concourse.tile as tile
    from concourse import bacc, mybir

    f32 = mybir.dt.float32
    bf16 = mybir.dt.bfloat16
    i32 = mybir.dt.int32
    Alu = mybir.AluOpType
    Act = mybir.ActivationFunctionType
    import bass_rust as _bass_rust

    class _Bacc1(bacc.Bacc):
        # Resolve every activation to table set 3 ("sqrt_and_others":
        # sqrt+square+copy) by blanking the other sets' func lists.  List
        # POSITION is the act_func_set_id, so ids stay correct.
        def insert_act_table_loads(self):
            from concourse.hw_specs import get_activation_tables

            has_activation = any(
                isinstance(i, mybir.InstActivation)
                for b in self.main_func.blocks
                for i in b.instructions
            )
            if not has_activation:
                return
            tables = [
                (n, (s if n == "sqrt_and_others" else set()))
                for n, s in get_activation_tables(self.m.arch).items()
            ]
            _bass_rust.insert_act_table_loads(self, tables)

    nc = _Bacc1("TRN2", target_bir_lowering=False, debug=False)
    inA = nc.dram_tensor("inA", [128, 512], bf16, kind="ExternalInput").ap()
    inB = nc.dram_tensor("inB", [128, 512], bf16, kind="ExternalInput").ap()
    out = nc.dram_tensor("out", [4, 1], f32, kind="ExternalOutput").ap()

    tinA = nc.alloc_sbuf_tensor("tinA", [128, 512], bf16).ap()
    tinB = nc.alloc_sbuf_tensor("tinB", [128, 512], bf16).ap()
    td = nc.alloc_sbuf_tensor("td", [128, 512], bf16).ap()
    dsq = nc.alloc_sbuf_tensor("dsq", [128, 512], bf16).ap()
    rc2 = nc.alloc_sbuf_tensor("rc2", [128, 2], f32).ap()
    rcs2 = nc.alloc_sbuf_tensor("rcs2", [128, 2], f32).ap()
    junk2 = nc.alloc_sbuf_tensor("junk2", [128, 2], f32).ap()
    m2rs = nc.alloc_sbuf_tensor("m2rs", [128, 1], f32).ap()
    sq2 = nc.alloc_sbuf_tensor("sq2", [128, 2], f32).ap()
    ti1 = nc.alloc_sbuf_tensor("ti1", [128, 4], i32).ap()
    ti1b = nc.alloc_sbuf_tensor("ti1b", [128, 4], i32).ap()
    mask01 = nc.alloc_sbuf_tensor("mask01", [128, 4], bf16).ap()
    mask01c = nc.alloc_sbuf_tensor("mask01c", [128, 4], bf16).ap()
    lhsT2 = nc.alloc_sbuf_tensor("lhsT2", [128, 4], bf16).ap()
    ti2 = nc.alloc_sbuf_tensor("ti2", [128, 4], i32).ap()
    ti2b = nc.alloc_sbuf_tensor("ti2b", [128, 4], i32).ap()
    maskS = nc.alloc_sbuf_tensor("maskS", [128, 4], f32).ap()
    ones4 = nc.alloc_sbuf_tensor("ones4", [4, 1], f32).ap()
    warm = nc.alloc_sbuf_tensor("warm", [4, 1], f32).ap()
    junkb = nc.alloc_sbuf_tensor("junkb", [4, 2], f32).ap()
    biasb = nc.alloc_sbuf_tensor("biasb", [4, 1], f32).ap()
    dist = nc.alloc_sbuf_tensor("dist", [4, 512], f32).ap()
    dsums = nc.alloc_sbuf_tensor("dsums", [4, 1], f32).ap()

    main = nc.alloc_psum_tensor("main", [4, 512], f32).ap()
    biasK2 = nc.alloc_psum_tensor("biasK2", [4, 2], f32).ap()

    tin0a, tin1a = tinA[:, 0:256], tinA[:, 256:512]
    tin0b, tin1b = tinB[:, 0:256], tinB[:, 256:512]
    td_a, td_b = td[:, 0:256], td[:, 256:512]
    dsq_a, dsq_b = dsq[:, 0:256], dsq[:, 256:512]
    rc_a, rc_b = rc2[:, 0:1], rc2[:, 1:2]
    sq_a, sq_b = sq2[:, 0:1], sq2[:, 1:2]
    main_a, main_b = main[:, 0:256], main[:, 256:512]

    with tile.TileContext(nc) as tc:
        # ---- packed input DMAs (pred_h | truth_h), A then B, SP queue ----
        with tc.high_priority():
            nc.sync.dma_start(tinA, inA)
            nc.scalar.dma_start(tinB, inB)

        # ---- dependency-free dummy Sqrt pulls the ACT table loads early ----
        nc.gpsimd.memset(ones4, 1.0)
        nc.scalar.activation(warm, ones4, Act.Sqrt)

        # ---- on-chip constants (under the DMA shadow) ----
        nc.gpsimd.iota(ti1, pattern=[[-1, 4]], base=0, channel_multiplier=1)
        nc.vector.tensor_scalar(ti1b, ti1, 3, None, op0=Alu.bitwise_and)
        # mask01[p,m] = 1.0 (bf16) iff p % 4 == m  -- dsq matmul lhsT
        nc.vector.tensor_scalar(mask01, ti1b, 0, None, op0=Alu.is_equal)
        # mask01c[p,m] = -2/4096 at the same positions -- lhsT2 seed
        nc.vector.tensor_scalar(mask01c, ti1b, 0, -2.0 / 4096.0,
                                op0=Alu.is_equal, op1=Alu.mult)
        nc.gpsimd.iota(ti2, pattern=[[-2, 2], [0, 2]], base=0,
                       channel_multiplier=1)
        nc.vector.tensor_scalar(ti2b, ti2, 1, 1, op0=Alu.arith_shift_right,
                                op1=Alu.bitwise_and)
        # maskS[p,m] = 1/1024 (f32) iff (p>>1)&1 == m//2  -- S2 group lhsT
        nc.vector.tensor_scalar(maskS, ti2b, 0, 1.0 / 1024.0,
                                op0=Alu.is_equal, op1=Alu.mult)

        # ---- per-half d = pred - truth; dsq_a on ScalarE (Square of
        #      0.5*d = d^2/4), dsq_b on DVE ----
        nc.vector.scalar_tensor_tensor(
            out=td_a, in0=tin0a, scalar=1.0, in1=tin1a,
            op0=Alu.mult, op1=Alu.subtract, accum_out=rc_a,
        )
        nc.scalar.activation(dsq_a, td_a, Act.Square, scale=0.5,
                             accum_out=sq_a)
        nc.vector.scalar_tensor_tensor(
            out=td_b, in0=tin0b, scalar=1.0, in1=tin1b,
            op0=Alu.mult, op1=Alu.subtract, accum_out=rc_b,
        )
        nc.vector.scalar_tensor_tensor(
            out=dsq_b, in0=td_b, scalar=0.25, in1=td_b,
            op0=Alu.mult, op1=Alu.mult, accum_out=sq_b,
        )

        # ---- rs chain (fused): rs[p] = rc_a+rc_b+rc_a[p^1]+rc_b[p^1] ----
        nc.vector.stream_shuffle(rcs2, rc2, mask=[i ^ 1 for i in range(32)])
        nc.vector.scalar_tensor_tensor(
            out=junk2, in0=rc2, scalar=1.0, in1=rcs2,
            op0=Alu.mult, op1=Alu.add, accum_out=m2rs,
        )
        # lhsT2[p,m] = (-2/4096) * rs[p] at group rows (bf16)
        nc.vector.scalar_tensor_tensor(
            out=lhsT2, in0=mask01c, scalar=m2rs, in1=mask01,
            op0=Alu.mult, op1=Alu.mult,
        )

        # ---- PE: one PSUM group in the main bank (start reset is
        #      bank-wide): d^2 halves open it, -2rs*d halves close it ----
        nc.tensor.matmul(main_a, mask01, dsq_a, start=True, stop=False,
                         skip_group_check=True)
        nc.tensor.matmul(main_b, mask01, dsq_b, start=False, stop=False,
                         skip_group_check=True)
        nc.tensor.matmul(biasK2, maskS, sq2, start=True, stop=True)
        nc.tensor.matmul(main_a, lhsT2, td_a, start=False, stop=False,
                         skip_group_check=True)
        nc.tensor.matmul(main_b, lhsT2, td_b, start=False, stop=True,
                         skip_group_check=True)

        # ---- biasb[m] = S2_g/4096 on the idle ScalarE (Copy + accum) ----
        nc.scalar.activation(junkb, biasK2, Act.Copy, accum_out=biasb)

        # ---- dist = sqrt(main + S2_g/4096); accum = row sums ----
        nc.scalar.activation(dist, main, Act.Sqrt, bias=biasb,
                             scale=1.0, accum_out=dsums)

        # ---- ship the 4 partial sums; host gather adds them ----
        nc.sync.dma_start(out, dsums)

    nc.compile()
    return nc


def _get():
    if "nc" not in _CACHE:
        _CACHE["nc"] = _build_nc()
    return _CACHE["nc"]


def _in_map(pred, truth):
    import ml_dtypes

    nc = _get()
    p = np.asarray(pred, dtype=np.float32).reshape(128, 512).astype(ml_dtypes.bfloat16)
    t = np.asarray(truth, dtype=np.float32).reshape(128, 512).astype(ml_dtypes.bfloat16)
    inA = np.concatenate([p[:, 0:256], t[:, 0:256]], axis=1)
    inB = np.concatenate([p[:, 256:512], t[:, 256:512]], axis=1)
    return nc, {
        "inA": np.ascontiguousarray(inA),
        "inB": np.ascontiguousarray(inB),
    }


def kernel(pred, truth) -> np.ndarray:
    from concourse.bass_utils import run_bass_kernel_spmd

    nc, in_map = _in_map(pred, truth)
    res = run_bass_kernel_spmd(
        nc, [dict(in_map) for _ in range(8)], core_ids=list(range(8))
    )
    total = res.results[0]["out"].astype(np.float64).sum()
    return np.asarray(total, dtype=np.float32).reshape(())


# revision 4
# speedup vs baseline: 1.0932x; 1.0572x over previous
"""Gcs pairwise-distance loss kernel (final, v16) for Trainium2 (Bass/Tile), 8-core SPMD.

Math: with d = pred - truth viewed as [128, 512] (partition p = 4b + k,
k in {0,1} = X column chunks, {2,3} = Y) and rs[p] = rc[p] + rc[p^1] the
full group-row sum:

    sumsq[m,j]/4096 = sum_{p%4==m} (d^2/4 - (2/4096) rs d)  +  S2_g/4096
    loss            = sum_{m,j} sqrt(sumsq[m,j]/4096)   (folds the /64)

v11 (vs v9): dsq_a moves to ScalarE as a Square activation (same ACT
table set as Sqrt), so DVE runs td_b as soon as half B lands instead of
after dsq_a; dsq_b stays on DVE for its cheap accumulator read.  Each column half arrives as ONE packed
[128,512] DMA (pred_h | truth_h side by side) — half A on the SP queue,
half B on the ACT queue, so the two descriptor pipelines and completion
sems process in parallel.  The d^2 matmuls run per column half in ONE PSUM
accumulation group (start only on the first matmul — the PSUM start reset
is bank-wide); the -2rs*d matmuls close the group.  The rs chain is fused
(one [128,2] shuffle + one accumulating stt + one stt for lhsT2).  The S2
bias merge rides the idle ScalarE as a Copy activation with accum_out
(both ACT tables are loaded up front regardless).  The Sqrt's accum_out
dsums[4,1] is DMA'd out; the host gather adds 4 floats.

Every core computes the full replicated result; core 0's output is returned.
"""

import numpy as np

_CACHE = {}


def _build_nc():
    import concourse.tile as tile
    from concourse import bacc, mybir

    f32 = mybir.dt.float32
    bf16 = mybir.dt.bfloat16
    i32 = mybir.dt.int32
    Alu = mybir.AluOpType
    Act = mybir.ActivationFunctionType
    import bass_rust as _bass_rust

    class _Bacc1(bacc.Bacc):
        # Resolve every activation to table set 3 ("sqrt_and_others":
        # sqrt+square+copy) by blanking the other sets' func lists.  List
        # POSITION is the act_func_set_id, so ids stay correct.
        def insert_act_table_loads(self):
            from concourse.hw_specs import get_activation_tables

            has_activation = any(
                isinstance(i, mybir.InstActivation)
                for b in self.main_func.blocks
                for i in b.instructions
            )
            if not has_activation:
                return
            tables = [
                (n, (s if n == "sqrt_and_others" else set()))
                for n, s in get_activation_tables(self.m.arch).items()
            ]
            _bass_rust.insert_act_table_loads(self, tables)

    nc = _Bacc1("TRN2", target_bir_lowering=False, debug=False)

    # ---- drop the three UNUSED const-AP memsets from the Bass preamble:
    #      gpsimd is the init-barrier straggler because of them ----
    _dead = ("const-float32-1.0", "const-bfloat16-1.0", "const-uint8-127")
    _blk = nc.main_func.blocks[0]
    for _i in [i for i in list(_blk.instructions)
               if "Memset" in type(i).__name__
               and any(d in str(i) for d in _dead)]:
        _blk.instructions.remove(_i)
    inA = nc.dram_tensor("inA", [128, 512], bf16, kind="ExternalInput").ap()
    inB = nc.dram_tensor("inB", [128, 512], bf16, kind="ExternalInput").ap()
    out = nc.dram_tensor("out", [4, 1], f32, kind="ExternalOutput").ap()

    tinA = nc.alloc_sbuf_tensor("tinA", [128, 512], bf16).ap()
    tinB = nc.alloc_sbuf_tensor("tinB", [128, 512], bf16).ap()
    td = nc.alloc_sbuf_tensor("td", [128, 512], bf16).ap()
    dsq = nc.alloc_sbuf_tensor("dsq", [128, 512], bf16).ap()
    rc2 = nc.alloc_sbuf_tensor("rc2", [128, 2], f32).ap()
    rcs2 = nc.alloc_sbuf_tensor("rcs2", [128, 2], f32).ap()
    junk2 = nc.alloc_sbuf_tensor("junk2", [128, 2], f32).ap()
    m2rs = nc.alloc_sbuf_tensor("m2rs", [128, 1], f32).ap()
    sq2 = nc.alloc_sbuf_tensor("sq2", [128, 2], f32).ap()
    ti1 = nc.alloc_sbuf_tensor("ti1", [128, 4], i32).ap()
    ti1b = nc.alloc_sbuf_tensor("ti1b", [128, 4], i32).ap()
    mask01 = nc.alloc_sbuf_tensor("mask01", [128, 4], bf16).ap()
    mask01c = nc.alloc_sbuf_tensor("mask01c", [128, 4], bf16).ap()
    lhsT2 = nc.alloc_sbuf_tensor("lhsT2", [128, 4], bf16).ap()
    ti2 = nc.alloc_sbuf_tensor("ti2", [128, 4], i32).ap()
    ti2b = nc.alloc_sbuf_tensor("ti2b", [128, 4], i32).ap()
    maskS = nc.alloc_sbuf_tensor("maskS", [128, 4], f32).ap()
    ones4 = nc.alloc_sbuf_tensor("ones4", [4, 1], f32).ap()
    warm = nc.alloc_sbuf_tensor("warm", [4, 1], f32).ap()
    junkb = nc.alloc_sbuf_tensor("junkb", [4, 2], f32).ap()
    biasb = nc.alloc_sbuf_tensor("biasb", [4, 1], f32).ap()
    dist = nc.alloc_sbuf_tensor("dist", [4, 512], f32).ap()
    dsums = nc.alloc_sbuf_tensor("dsums", [4, 1], f32).ap()

    main = nc.alloc_psum_tensor("main", [4, 512], f32).ap()
    biasK2 = nc.alloc_psum_tensor("biasK2", [4, 2], f32).ap()

    tin0a, tin1a = tinA[:, 0:256], tinA[:, 256:512]
    tin0b, tin1b = tinB[:, 0:256], tinB[:, 256:512]
    td_a, td_b = td[:, 0:256], td[:, 256:512]
    dsq_a, dsq_b = dsq[:, 0:256], dsq[:, 256:512]
    rc_a, rc_b = rc2[:, 0:1], rc2[:, 1:2]
    sq_a, sq_b = sq2[:, 0:1], sq2[:, 1:2]
    main_a, main_b = main[:, 0:256], main[:, 256:512]

    with tile.TileContext(nc) as tc:
        # ---- packed input DMAs (pred_h | truth_h), A then B, SP queue ----
        with tc.high_priority():
            nc.sync.dma_start(tinA, inA)
            nc.scalar.dma_start(tinB, inB)

        # ---- dependency-free dummy Sqrt pulls the ACT table loads early ----
        nc.gpsimd.memset(ones4, 1.0)
        nc.scalar.activation(warm, ones4, Act.Sqrt)

        # ---- on-chip constants (under the DMA shadow) ----
        nc.gpsimd.iota(ti1, pattern=[[-1, 4]], base=0, channel_multiplier=1)
        nc.vector.tensor_scalar(ti1b, ti1, 3, None, op0=Alu.bitwise_and)
        # mask01[p,m] = 1.0 (bf16) iff p % 4 == m  -- dsq matmul lhsT
        nc.vector.tensor_scalar(mask01, ti1b, 0, None, op0=Alu.is_equal)
        # mask01c[p,m] = -2/4096 at the same positions -- lhsT2 seed
        nc.vector.tensor_scalar(mask01c, ti1b, 0, -2.0 / 4096.0,
                                op0=Alu.is_equal, op1=Alu.mult)
        nc.gpsimd.iota(ti2, pattern=[[-2, 2], [0, 2]], base=0,
                       channel_multiplier=1)
        nc.vector.tensor_scalar(ti2b, ti2, 1, 1, op0=Alu.arith_shift_right,
                                op1=Alu.bitwise_and)
        # maskS[p,m] = 1/1024 (f32) iff (p>>1)&1 == m//2  -- S2 group lhsT
        nc.vector.tensor_scalar(maskS, ti2b, 0, 1.0 / 1024.0,
                                op0=Alu.is_equal, op1=Alu.mult)

        # ---- per-half d = pred - truth; dsq_a on ScalarE (Square of
        #      0.5*d = d^2/4), dsq_b on DVE ----
        nc.vector.scalar_tensor_tensor(
            out=td_a, in0=tin0a, scalar=1.0, in1=tin1a,
            op0=Alu.mult, op1=Alu.subtract, accum_out=rc_a,
        )
        nc.scalar.activation(dsq_a, td_a, Act.Square, scale=0.5,
                             accum_out=sq_a)
        nc.vector.scalar_tensor_tensor(
            out=td_b, in0=tin0b, scalar=1.0, in1=tin1b,
            op0=Alu.mult, op1=Alu.subtract, accum_out=rc_b,
        )
        nc.vector.scalar_tensor_tensor(
            out=dsq_b, in0=td_b, scalar=0.25, in1=td_b,
            op0=Alu.mult, op1=Alu.mult, accum_out=sq_b,
        )

        # ---- rs chain (fused): rs[p] = rc_a+rc_b+rc_a[p^1]+rc_b[p^1] ----
        nc.vector.stream_shuffle(rcs2, rc2, mask=[i ^ 1 for i in range(32)])
        nc.vector.scalar_tensor_tensor(
            out=junk2, in0=rc2, scalar=1.0, in1=rcs2,
            op0=Alu.mult, op1=Alu.add, accum_out=m2rs,
        )
        # lhsT2[p,m] = (-2/4096) * rs[p] at group rows (bf16)
        nc.vector.scalar_tensor_tensor(
            out=lhsT2, in0=mask01c, scalar=m2rs, in1=mask01,
            op0=Alu.mult, op1=Alu.mult,
        )

        # ---- PE: one PSUM group in the main bank (start reset is
        #      bank-wide): d^2 halves open it, -2rs*d halves close it ----
        nc.tensor.matmul(main_a, mask01, dsq_a, start=True, stop=False,
                         skip_group_check=True)
        nc.tensor.matmul(main_b, mask01, dsq_b, start=False, stop=False,
                         skip_group_check=True)
        nc.tensor.matmul(biasK2, maskS, sq2, start=True, stop=True)
        nc.tensor.matmul(main_a, lhsT2, td_a, start=False, stop=False,
                         skip_group_check=True)
        nc.tensor.matmul(main_b, lhsT2, td_b, start=False, stop=True,
                         skip_group_check=True)

        # ---- biasb[m] = S2_g/4096 on the idle ScalarE (Copy + accum) ----
        nc.scalar.activation(junkb, biasK2, Act.Copy, accum_out=biasb)

        # ---- dist = sqrt(main + S2_g/4096); accum = row sums ----
        nc.scalar.activation(dist, main, Act.Sqrt, bias=biasb,
                             scale=1.0, accum_out=dsums)

        # ---- ship the 4 partial sums; host gather adds them ----
        nc.sync.dma_start(out, dsums)

    nc.compile()
    return nc


def _get():
    if "nc" not in _CACHE:
        _CACHE["nc"] = _build_nc()
    return _CACHE["nc"]


def _in_map(pred, truth):
    import ml_dtypes

    nc = _get()
    p = np.asarray(pred, dtype=np.float32).reshape(128, 512).astype(ml_dtypes.bfloat16)
    t = np.asarray(truth, dtype=np.float32).reshape(128, 512).astype(ml_dtypes.bfloat16)
    inA = np.concatenate([p[:, 0:256], t[:, 0:256]], axis=1)
    inB = np.concatenate([p[:, 256:512], t[:, 256:512]], axis=1)
    return nc, {
        "inA": np.ascontiguousarray(inA),
        "inB": np.ascontiguousarray(inB),
    }


def kernel(pred, truth) -> np.ndarray:
    from concourse.bass_utils import run_bass_kernel_spmd

    nc, in_map = _in_map(pred, truth)
    res = run_bass_kernel_spmd(
        nc, [dict(in_map) for _ in range(8)], core_ids=list(range(8))
    )
    total = res.results[0]["out"].astype(np.float64).sum()
    return np.asarray(total, dtype=np.float32).reshape(())


# revision 5
# speedup vs baseline: 1.2986x; 1.1879x over previous
"""Gcs pairwise-distance loss kernel (final, v17) for Trainium2 (Bass/Tile), 8-core SPMD.

Math: with d = pred - truth viewed as [128, 512] (partition p = 4b + k,
k in {0,1} = X column chunks, {2,3} = Y) and rs[p] = rc[p] + rc[p^1] the
full group-row sum:

    sumsq[m,j]/4096 = sum_{p%4==m} (d^2/4 - (2/4096) rs d)  +  S2_g/4096
    loss            = sum_{m,j} sqrt(sumsq[m,j]/4096)   (folds the /64)

v11 (vs v9): dsq_a moves to ScalarE as a Square activation (same ACT
table set as Sqrt), so DVE runs td_b as soon as half B lands instead of
after dsq_a; dsq_b stays on DVE for its cheap accumulator read.  Each column half arrives as ONE packed
[128,512] DMA (pred_h | truth_h side by side) — half A on the SP queue,
half B on the ACT queue, so the two descriptor pipelines and completion
sems process in parallel.  The d^2 matmuls run per column half in ONE PSUM
accumulation group (start only on the first matmul — the PSUM start reset
is bank-wide); the -2rs*d matmuls close the group.  The rs chain is fused
(one [128,2] shuffle + one accumulating stt + one stt for lhsT2).  The S2
bias merge rides the idle ScalarE as a Copy activation with accum_out
(both ACT tables are loaded up front regardless).  The Sqrt's accum_out
dsums[4,1] is DMA'd out; the host gather adds 4 floats.

Every core computes the full replicated result; core 0's output is returned.
"""

import numpy as np

_CACHE = {}


def _build_nc():
    import concourse.tile as tile
    from concourse import bacc, mybir

    f32 = mybir.dt.float32
    bf16 = mybir.dt.bfloat16
    i32 = mybir.dt.int32
    Alu = mybir.AluOpType
    Act = mybir.ActivationFunctionType
    import bass_rust as _bass_rust

    class _Bacc1(bacc.Bacc):
        # Resolve every activation to table set 3 ("sqrt_and_others":
        # sqrt+square+copy) by blanking the other sets' func lists.  List
        # POSITION is the act_func_set_id, so ids stay correct.
        def insert_act_table_loads(self):
            from concourse.hw_specs import get_activation_tables

            has_activation = any(
                isinstance(i, mybir.InstActivation)
                for b in self.main_func.blocks
                for i in b.instructions
            )
            if not has_activation:
                return
            tables = [
                (n, (s if n == "sqrt_and_others" else set()))
                for n, s in get_activation_tables(self.m.arch).items()
            ]
            _bass_rust.insert_act_table_loads(self, tables)

    nc = _Bacc1("TRN2", target_bir_lowering=False, debug=False)

    # ---- drop the three UNUSED const-AP memsets from the Bass preamble:
    #      gpsimd is the init-barrier straggler because of them ----
    _dead = ("const-float32-1.0", "const-bfloat16-1.0", "const-uint8-127",
             "const-float32-0.0")
    _blk = nc.main_func.blocks[0]
    for _i in [i for i in list(_blk.instructions)
               if "Memset" in type(i).__name__
               and any(d in str(i) for d in _dead)]:
        _blk.instructions.remove(_i)
    inA = nc.dram_tensor("inA", [128, 512], bf16, kind="ExternalInput").ap()
    inB = nc.dram_tensor("inB", [128, 512], bf16, kind="ExternalInput").ap()
    out = nc.dram_tensor("out", [4, 1], f32, kind="ExternalOutput").ap()

    tinA = nc.alloc_sbuf_tensor("tinA", [128, 512], bf16).ap()
    tinB = nc.alloc_sbuf_tensor("tinB", [128, 512], bf16).ap()
    td = nc.alloc_sbuf_tensor("td", [128, 512], bf16).ap()
    dsq = nc.alloc_sbuf_tensor("dsq", [128, 512], bf16).ap()
    rc2 = nc.alloc_sbuf_tensor("rc2", [128, 2], f32).ap()
    rcs2 = nc.alloc_sbuf_tensor("rcs2", [128, 2], f32).ap()
    junk2 = nc.alloc_sbuf_tensor("junk2", [128, 2], f32).ap()
    m2rs = nc.alloc_sbuf_tensor("m2rs", [128, 1], f32).ap()
    sq2 = nc.alloc_sbuf_tensor("sq2", [128, 2], f32).ap()
    ti1 = nc.alloc_sbuf_tensor("ti1", [128, 4], i32).ap()
    ti1b = nc.alloc_sbuf_tensor("ti1b", [128, 4], i32).ap()
    mask01 = nc.alloc_sbuf_tensor("mask01", [128, 4], bf16).ap()
    mask01c = nc.alloc_sbuf_tensor("mask01c", [128, 4], bf16).ap()
    lhsT2 = nc.alloc_sbuf_tensor("lhsT2", [128, 4], bf16).ap()
    ti2 = nc.alloc_sbuf_tensor("ti2", [128, 4], i32).ap()
    ti2b = nc.alloc_sbuf_tensor("ti2b", [128, 4], i32).ap()
    maskS = nc.alloc_sbuf_tensor("maskS", [128, 4], f32).ap()
    ones4 = nc.alloc_sbuf_tensor("ones4", [4, 1], f32).ap()
    z128 = nc.alloc_sbuf_tensor("z128", [128, 1], f32).ap()
    z42 = nc.alloc_sbuf_tensor("z42", [4, 2], f32).ap()
    warm = nc.alloc_sbuf_tensor("warm", [4, 1], f32).ap()
    junkb = nc.alloc_sbuf_tensor("junkb", [4, 2], f32).ap()
    biasb = nc.alloc_sbuf_tensor("biasb", [4, 1], f32).ap()
    dist = nc.alloc_sbuf_tensor("dist", [4, 512], f32).ap()
    dsums = nc.alloc_sbuf_tensor("dsums", [4, 1], f32).ap()

    main = nc.alloc_psum_tensor("main", [4, 512], f32).ap()
    biasK2 = nc.alloc_psum_tensor("biasK2", [4, 2], f32).ap()

    tin0a, tin1a = tinA[:, 0:256], tinA[:, 256:512]
    tin0b, tin1b = tinB[:, 0:256], tinB[:, 256:512]
    td_a, td_b = td[:, 0:256], td[:, 256:512]
    dsq_a, dsq_b = dsq[:, 0:256], dsq[:, 256:512]
    rc_a, rc_b = rc2[:, 0:1], rc2[:, 1:2]
    sq_a, sq_b = sq2[:, 0:1], sq2[:, 1:2]
    main_a, main_b = main[:, 0:256], main[:, 256:512]

    with tile.TileContext(nc) as tc:
        # ---- packed input DMAs (pred_h | truth_h), A then B, SP queue ----
        with tc.high_priority():
            nc.sync.dma_start(tinA, inA)
            nc.scalar.dma_start(tinB, inB)

        # ---- dependency-free dummy Sqrt pulls the ACT table loads early ----
        nc.gpsimd.memset(ones4, 1.0)
        nc.gpsimd.memset(z128, 0.0)
        nc.gpsimd.memset(z42, 0.0)
        nc.scalar.activation(warm, ones4, Act.Sqrt, bias=z128[0:4, :])

        # ---- on-chip constants (under the DMA shadow) ----
        nc.gpsimd.iota(ti1, pattern=[[-1, 4]], base=0, channel_multiplier=1)
        nc.vector.tensor_scalar(ti1b, ti1, 3, None, op0=Alu.bitwise_and)
        # mask01[p,m] = 1.0 (bf16) iff p % 4 == m  -- dsq matmul lhsT
        nc.vector.tensor_scalar(mask01, ti1b, 0, None, op0=Alu.is_equal)
        # mask01c[p,m] = -2/4096 at the same positions -- lhsT2 seed
        nc.vector.tensor_scalar(mask01c, ti1b, 0, -2.0 / 4096.0,
                                op0=Alu.is_equal, op1=Alu.mult)
        nc.gpsimd.iota(ti2, pattern=[[-2, 2], [0, 2]], base=0,
                       channel_multiplier=1)
        nc.vector.tensor_scalar(ti2b, ti2, 1, 1, op0=Alu.arith_shift_right,
                                op1=Alu.bitwise_and)
        # maskS[p,m] = 1/1024 (f32) iff (p>>1)&1 == m//2  -- S2 group lhsT
        nc.vector.tensor_scalar(maskS, ti2b, 0, 1.0 / 1024.0,
                                op0=Alu.is_equal, op1=Alu.mult)

        # ---- per-half d = pred - truth; dsq_a on ScalarE (Square of
        #      0.5*d = d^2/4), dsq_b on DVE ----
        nc.vector.scalar_tensor_tensor(
            out=td_a, in0=tin0a, scalar=1.0, in1=tin1a,
            op0=Alu.mult, op1=Alu.subtract, accum_out=rc_a,
        )
        nc.scalar.activation(dsq_a, td_a, Act.Square, scale=0.5,
                             bias=z128, accum_out=sq_a)
        nc.vector.scalar_tensor_tensor(
            out=td_b, in0=tin0b, scalar=1.0, in1=tin1b,
            op0=Alu.mult, op1=Alu.subtract, accum_out=rc_b,
        )
        nc.vector.scalar_tensor_tensor(
            out=dsq_b, in0=td_b, scalar=0.25, in1=td_b,
            op0=Alu.mult, op1=Alu.mult, accum_out=sq_b,
        )

        # ---- rs chain (fused): rs[p] = rc_a+rc_b+rc_a[p^1]+rc_b[p^1] ----
        nc.vector.stream_shuffle(rcs2, rc2, mask=[i ^ 1 for i in range(32)])
        nc.vector.scalar_tensor_tensor(
            out=junk2, in0=rc2, scalar=1.0, in1=rcs2,
            op0=Alu.mult, op1=Alu.add, accum_out=m2rs,
        )
        # lhsT2[p,m] = (-2/4096) * rs[p] at group rows (bf16)
        nc.vector.scalar_tensor_tensor(
            out=lhsT2, in0=mask01c, scalar=m2rs, in1=mask01,
            op0=Alu.mult, op1=Alu.mult,
        )

        # ---- PE: one PSUM group in the main bank (start reset is
        #      bank-wide): d^2 halves open it, -2rs*d halves close it ----
        nc.tensor.matmul(main_a, mask01, dsq_a, start=True, stop=False,
                         skip_group_check=True)
        nc.tensor.matmul(main_b, mask01, dsq_b, start=False, stop=False,
                         skip_group_check=True)
        nc.tensor.matmul(biasK2, maskS, sq2, start=True, stop=True)
        nc.tensor.matmul(main_a, lhsT2, td_a, start=False, stop=False,
                         skip_group_check=True)
        nc.tensor.matmul(main_b, lhsT2, td_b, start=False, stop=True,
                         skip_group_check=True)

        # ---- biasb[m] = S2_g/4096 on the idle DVE (stt + accum) ----
        nc.vector.scalar_tensor_tensor(
            out=junkb, in0=biasK2, scalar=1.0, in1=z42,
            op0=Alu.mult, op1=Alu.add, accum_out=biasb,
        )

        # ---- dist = sqrt(main + S2_g/4096); accum = row sums ----
        nc.scalar.activation(dist, main, Act.Sqrt, bias=biasb,
                             scale=1.0, accum_out=dsums)

        # ---- ship the 4 partial sums; host gather adds them ----
        nc.sync.dma_start(out, dsums)

    nc.compile()
    return nc


def _get():
    if "nc" not in _CACHE:
        _CACHE["nc"] = _build_nc()
    return _CACHE["nc"]


def _in_map(pred, truth):
    import ml_dtypes

    nc = _get()
    p = np.asarray(pred, dtype=np.float32).reshape(128, 512).astype(ml_dtypes.bfloat16)
    t = np.asarray(truth, dtype=np.float32).reshape(128, 512).astype(ml_dtypes.bfloat16)
    inA = np.concatenate([p[:, 0:256], t[:, 0:256]], axis=1)
    inB = np.concatenate([p[:, 256:512], t[:, 256:512]], axis=1)
    return nc, {
        "inA": np.ascontiguousarray(inA),
        "inB": np.ascontiguousarray(inB),
    }


def kernel(pred, truth) -> np.ndarray:
    from concourse.bass_utils import run_bass_kernel_spmd

    nc, in_map = _in_map(pred, truth)
    res = run_bass_kernel_spmd(
        nc, [dict(in_map) for _ in range(8)], core_ids=list(range(8))
    )
    total = res.results[0]["out"].astype(np.float64).sum()
    return np.asarray(total, dtype=np.float32).reshape(())
